# revision 37
# baseline (speedup 1.0000x reference)
"""Trainium2 Bass kernel: dense transformer block (B=2, T=2048, C=1024, H=16, HD=64).

Sharding over 8 NeuronCores: 2 batch groups (data parallel over B) x 4-way
tensor parallel within each group. Per group of 4 cores:
  - attention: heads split 4 ways (4 heads/core); per-core proj partials are
    ReduceScatter'd (bf16) over the token axis.
  - FFN: token-split (512 tokens/core), full W1/W2 streamed per core in bf16.

v2: fused chunk pipeline (LN1 -> QKV -> attention -> proj -> RS per 512-token
q-chunk), LN gains/biases folded into weights on the host, bf16 PE transposes,
merged QKV head-pair matmuls, AV matmul with a fused ones-column rowsum,
FFN with split W1 (fills the last-RS window) and windowed PSUM spill so the
whole W2 accumulation fits 8 banks with no second pass.
"""

import os
import sys

if "/opt/trn_rl_repo" not in sys.path:
    sys.path.insert(0, "/opt/trn_rl_repo")

import contextlib
import math

import ml_dtypes
import numpy as np

import concourse.bass as bass
import concourse.mybir as mybir
import concourse.tile as tile
from concourse import bacc
from concourse.bass_utils import run_bass_kernel_spmd
from concourse.masks import make_identity

# bass_utils' trace path imports antenv.axon_hooks, absent in this container.
try:
    from antenv import axon_hooks as _ah  # noqa: F401
except ImportError:
    import types as _types

    _shim = _types.ModuleType("antenv.axon_hooks")
    _shim._hook = None
    _shim.set_axon_ntff_profile_hook = lambda h: setattr(_shim, "_hook", h)
    _shim.get_axon_ntff_profile_hook = lambda: _shim._hook
    sys.modules["antenv.axon_hooks"] = _shim
    try:
        if "/root/.axon_site" not in sys.path:
            sys.path.insert(0, "/root/.axon_site")
        from trn_agent_boot.trn_boot import _ntff_profile_via_ctypes

        _shim.set_axon_ntff_profile_hook(
            _ntff_profile_via_ctypes("/opt/axon/libaxon_pjrt.so")
        )
    except Exception:
        pass

AF = mybir.ActivationFunctionType
ALU = mybir.AluOpType
FP32 = mybir.dt.float32
BF16 = mybir.dt.bfloat16

P = 128
QCH = 512  # query chunk (free dim of S^T matmuls)
KG = 2  # k-tiles batched per exp() call
GA = 13  # hidden tiles whose W1 token-prefix runs during the last RS


def build_block(T=2048, C=1024, NHL=4, F=4096, GC=4, eps=1e-5, n_cores=8):
    """Emit the per-core SPMD program. NHL = local heads (64-dim each)."""
    HD = 64
    DL = NHL * HD  # 256
    NPAIR = NHL // 2
    NT = T // P  # token tiles (16)
    NCc = C // P  # channel tiles (8)
    NQC = T // QCH  # query chunks (4)
    KPC = QCH // P  # k-tiles per chunk (4)
    TPC = NT // NQC  # token tiles per chunk (4)
    TSH = T // GC  # token shard (512)
    NST = TSH // P  # shard token tiles (4)
    NHT = F // P  # FFN hidden tiles (32)
    W1A = (NST - 1) * P  # W1 token prefix computable before last RS (384)
    scale = 1.0 / math.sqrt(HD)

    groups = [list(range(g * GC, (g + 1) * GC)) for g in range(n_cores // GC)]

    nc = bacc.Bacc(
        "TRN2", target_bir_lowering=False, num_devices=n_cores, debug=False
    )

    # ---- I/O ----
    x_full = nc.dram_tensor("x_full", [T, C], BF16, kind="ExternalInput")
    x_shard = nc.dram_tensor("x_shard", [TSH, C], FP32, kind="ExternalInput")
    wq_d = nc.dram_tensor("wq", [C, DL], BF16, kind="ExternalInput")
    wk_d = nc.dram_tensor("wk", [C, DL], BF16, kind="ExternalInput")
    wv_d = nc.dram_tensor("wv", [C, DL], BF16, kind="ExternalInput")
    wp_d = nc.dram_tensor("wp", [DL, C], BF16, kind="ExternalInput")
    w1_d = nc.dram_tensor("w1", [C, F], BF16, kind="ExternalInput")
    w2_d = nc.dram_tensor("w2", [F, C], BF16, kind="ExternalInput")
    qkb_d = nc.dram_tensor("qkb", [P, 2 * NPAIR * 2], FP32, kind="ExternalInput")
    vb_d = nc.dram_tensor("vb", [NHL * (HD + 1)], BF16, kind="ExternalInput")
    b1_d = nc.dram_tensor("b1r", [P, NHT], FP32, kind="ExternalInput")
    bp4_d = nc.dram_tensor("bp4", [C], BF16, kind="ExternalInput")
    b2_d = nc.dram_tensor("b2", [C], BF16, kind="ExternalInput")
    mask_d = nc.dram_tensor("maskr", [P, P], BF16, kind="ExternalInput")
    out_d = nc.dram_tensor("out", [TSH, C], BF16, kind="ExternalOutput")

    def bcast(dst, src_ap):
        """DMA a [N]-shaped dram AP broadcast onto [P, N] sbuf tile."""
        nc.sync.dma_start(
            dst,
            bass.AP(
                tensor=src_ap.tensor,
                offset=src_ap.offset,
                ap=[[0, P]] + list(src_ap.ap),
            ),
        )

    with tile.TileContext(nc) as tc, contextlib.ExitStack() as est:
        sing = est.enter_context(tc.tile_pool(name="sing", bufs=1))
        big = est.enter_context(tc.tile_pool(name="big", bufs=1))
        tok = est.enter_context(tc.tile_pool(name="tok", bufs=5))
        xp = est.enter_context(tc.tile_pool(name="xp", bufs=8))
        vp = est.enter_context(tc.tile_pool(name="vp", bufs=NT))
        attp = est.enter_context(tc.tile_pool(name="attp", bufs=2 * NPAIR))
        exps = est.enter_context(tc.tile_pool(name="exps", bufs=3))
        rsp = est.enter_context(tc.tile_pool(name="rsp", bufs=4))
        recp = est.enter_context(tc.tile_pool(name="recp", bufs=2))
        pjp = est.enter_context(tc.tile_pool(name="pjp", bufs=4))
        x2p = est.enter_context(tc.tile_pool(name="x2p", bufs=NST))
        w1p = est.enter_context(tc.tile_pool(name="w1p", bufs=GA))
        w2p = est.enter_context(tc.tile_pool(name="w2p", bufs=6))
        w2bp = est.enter_context(tc.tile_pool(name="w2bp", bufs=4))
        accp = est.enter_context(tc.tile_pool(name="accp", bufs=1))
        obp = est.enter_context(tc.tile_pool(name="obp", bufs=3))
        dram = est.enter_context(tc.tile_pool(name="dram", bufs=1, space="DRAM"))

        # ---- x tile prefetch: issue DMAs well before the collectives start
        # hogging the fabric (a ReduceScatter starves ordinary DMA traffic).
        x_tiles = {}

        def fetch_x(it):
            if it in x_tiles:
                return
            t = xp.tile([P, C], BF16, tag="x", name=f"xt{it}")
            nc.sync.dma_start(t, x_full[it * P : (it + 1) * P, :])
            x_tiles[it] = t

        for it in range(2 * TPC):  # chunks 0+1 up front
            fetch_x(it)

        ident = sing.tile([P, P], BF16, tag="ident", name="ident")
        make_identity(nc, ident)
        eps_t = sing.tile([P, 1], FP32, tag="eps", name="eps")
        nc.vector.memset(eps_t, eps)
        ones_t = sing.tile([1, HD], BF16, tag="ones", name="ones")
        nc.vector.memset(ones_t, 1.0)

        wq_sb = sing.tile([P, NCc, DL], BF16, tag="wq", name="wq")
        nc.sync.dma_start(wq_sb, wq_d.rearrange("(n p) m -> p n m", p=P))
        wk_sb = sing.tile([P, NCc, DL], BF16, tag="wk", name="wk")
        nc.sync.dma_start(wk_sb, wk_d.rearrange("(n p) m -> p n m", p=P))
        wv_sb = sing.tile([P, NCc, DL], BF16, tag="wv", name="wv")
        nc.sync.dma_start(wv_sb, wv_d.rearrange("(n p) m -> p n m", p=P))
        wp_sb = sing.tile([P, DL // P, C], BF16, tag="wp", name="wp")
        nc.sync.dma_start(wp_sb, wp_d.rearrange("(n p) m -> p n m", p=P))
        mask_sb = sing.tile([P, P], BF16, tag="mask", name="mask")
        nc.sync.dma_start(mask_sb, mask_d[:])
        qkb_sb = sing.tile([P, 2 * NPAIR * 2], FP32, tag="qkb", name="qkb")
        nc.sync.dma_start(qkb_sb, qkb_d[:])
        vb_sb = sing.tile([P, NHL, HD + 1], BF16, tag="vb", name="vb")
        bcast(vb_sb.rearrange("p h d -> p (h d)"), vb_d[:])
        b1_sb = sing.tile([P, NHT], FP32, tag="b1", name="b1")
        nc.sync.dma_start(b1_sb, b1_d[:])
        bp4_bc = sing.tile([P, C], BF16, tag="bp4", name="bp4")
        bcast(bp4_bc, bp4_d[:])
        b2_bc = sing.tile([P, C], BF16, tag="b2bc", name="b2bc")
        bcast(b2_bc, b2_d[:])

        # hT (attention input, feature-major, strided by token tile) shares its
        # 32KB/partition slot with hidT (FFN hidden) — disjoint lifetimes.
        hT = big.tile([P, NT, NCc, P], BF16, tag="big", name="hT")
        QT = [sing.tile([P, T], BF16, tag=f"qt{pr}", name=f"qt{pr}") for pr in range(NPAIR)]
        KT = [sing.tile([P, T], BF16, tag=f"kt{pr}", name=f"kt{pr}") for pr in range(NPAIR)]
        h2T = sing.tile([P, NST, NCc, P], BF16, tag="h2T", name="h2T")

        # chunks 0+1 share one ReduceScatter (fewer collective launches);
        # chunks 2 and 3 keep their own so the tail RS stays small.
        rs_in_t = [
            dram.tile([2 * QCH, C], BF16, tag="rsinA", name="rsinA"),
            dram.tile([QCH, C], BF16, tag="rsin2", name="rsin2"),
            dram.tile([QCH, C], BF16, tag="rsin3", name="rsin3"),
        ]
        rs_out_t = [
            dram.tile([2 * QCH // GC, C], BF16, tag="rsoutA", name="rsoutA"),
            dram.tile([QCH // GC, C], BF16, tag="rsout2", name="rsout2"),
            dram.tile([QCH // GC, C], BF16, tag="rsout3", name="rsout3"),
        ]

        # ---- LayerNorm stats helper (token-major [P, C] tile -> mv[P,2]) ----
        fmax = math.gcd(512, C)
        nsub = C // fmax

        def ln_stats(x_t, pool, tagp):
            stats = pool.tile([P, nsub, 6], FP32, tag=f"{tagp}_st", name=f"{tagp}_st")
            xr = x_t.rearrange("p (n f) -> p n f", n=nsub)
            for s in range(nsub):
                nc.vector.bn_stats(out=stats[:, s, :], in_=xr[:, s, :])
            mv = pool.tile([P, 2], FP32, tag=f"{tagp}_mv", name=f"{tagp}_mv")
            nc.vector.bn_aggr(out=mv, in_=stats)
            nc.scalar.activation(
                out=mv[:, 1:2], in_=mv[:, 1:2], func=AF.Sqrt, bias=eps_t, scale=1.0
            )
            nc.vector.reciprocal(out=mv[:, 1:2], in_=mv[:, 1:2])
            return mv

        V4 = [None] * NT

        with contextlib.ExitStack() as ps_est:
            psMain = ps_est.enter_context(tc.tile_pool(name="psMain", bufs=2, space="PSUM"))
            psS = ps_est.enter_context(tc.tile_pool(name="psS", bufs=2, space="PSUM"))
            psX = ps_est.enter_context(tc.tile_pool(name="psX", bufs=2, space="PSUM"))

            def phase_avqkv(ch):
                """LN1 + transpose + V + QKV for one 512-token chunk."""
                if ch + 2 < NQC:  # stage next-next chunk's inputs now
                    for tt in range(TPC):
                        fetch_x((ch + 2) * TPC + tt)
                for tt in range(TPC):
                    it = ch * TPC + tt
                    x_t = x_tiles.pop(it)
                    mv = ln_stats(x_t, tok, "ln1")
                    h_bf = tok.tile([P, C], BF16, tag="tb", name=f"hbf{it}")
                    nc.vector.tensor_scalar(
                        out=h_bf,
                        in0=x_t,
                        scalar1=mv[:, 0:1],
                        scalar2=mv[:, 1:2],
                        op0=ALU.subtract,
                        op1=ALU.mult,
                    )
                    trans = psX.tile([P, NCc * P], BF16, tag="x", name=f"tr{it}")
                    for ic in range(NCc):
                        nc.tensor.transpose(
                            trans[:, ic * P : (ic + 1) * P],
                            h_bf[:, ic * P : (ic + 1) * P],
                            ident,
                        )
                    nc.vector.tensor_copy(
                        hT[:, it, :, :].rearrange("p a b -> p (a b)"), trans
                    )

                for tt in range(TPC):
                    it = ch * TPC + tt
                    ps = psMain.tile([P, DL], FP32, tag="m", name=f"vps{it}")
                    for ic in range(NCc):
                        nc.tensor.matmul(
                            ps,
                            lhsT=hT[:, it, ic, :],
                            rhs=wv_sb[:, ic, :],
                            start=(ic == 0),
                            stop=(ic == NCc - 1),
                        )
                    v_t = vp.tile([P, NHL, HD + 1], BF16, tag="v", name=f"v{it}")
                    nc.vector.tensor_tensor(
                        out=v_t[:, :, 0:HD],
                        in0=ps.rearrange("p (h d) -> p h d", h=NHL),
                        in1=vb_sb[:, :, 0:HD],
                        op=ALU.add,
                    )
                    nc.vector.tensor_copy(v_t[:, :, HD : HD + 1], vb_sb[:, :, HD : HD + 1])
                    V4[it] = v_t

                for pr in range(NPAIR):
                    for di, (dst, w_sb) in enumerate(((QT, wq_sb), (KT, wk_sb))):
                        ps = psMain.tile([P, QCH], FP32, tag="m", name=f"qk{pr}{di}")
                        for ic in range(NCc):
                            nc.tensor.matmul(
                                ps,
                                lhsT=w_sb[:, ic, pr * P : (pr + 1) * P],
                                rhs=hT[:, ch * TPC : (ch + 1) * TPC, ic, :],
                                start=(ic == 0),
                                stop=(ic == NCc - 1),
                            )
                        col = di * NPAIR + pr
                        dseg = dst[pr][:, ch * QCH : (ch + 1) * QCH]
                        nc.scalar.activation(
                            out=dseg, in_=ps, func=AF.Identity,
                            bias=qkb_sb[:, col : col + 1], scale=1.0,
                        )

            # software pipeline: LN/QKV of chunk ch+1 runs during attention(ch)
            phase_avqkv(0)
            for ch in range(NQC):
                if ch + 1 < NQC:
                    phase_avqkv(ch + 1)

                # ===== attention for this chunk =====
                n_kt = (ch + 1) * KPC
                attT = []
                for pr in range(NPAIR):
                    att_ps = [
                        psMain.tile([P, QCH], FP32, tag="m", name=f"av{ch}{pr}{i}")
                        for i in range(2)
                    ]
                    for kg in range(n_kt // KG):
                        s_ps = [
                            psS.tile([P, KG * QCH], FP32, tag="s", name=f"s{pr}{kg}{i}")
                            for i in range(2)
                        ]

                        def vis(kt):
                            """first causally-visible q column for this k-tile"""
                            jd = kt - ch * KPC
                            return jd * P if 0 < jd < KPC else 0

                        for j in range(KG):
                            kt = kg * KG + j
                            for pos in range(2):
                                nc.tensor.matmul(
                                    s_ps[pos][:, j * QCH + vis(kt) : (j + 1) * QCH],
                                    lhsT=KT[pr][
                                        64 * pos : 64 * pos + 64, kt * P : (kt + 1) * P
                                    ],
                                    rhs=QT[pr][
                                        64 * pos : 64 * pos + 64,
                                        ch * QCH + vis(kt) : (ch + 1) * QCH,
                                    ],
                                    start=True,
                                    stop=True,
                                    tile_position=(64 * pos, 0),
                                )
                        for pos in range(2):
                            e_sb = exps.tile([P, KG * QCH], BF16, tag="e", name="e")
                            for j in range(KG):
                                kt = kg * KG + j
                                lo = j * QCH + vis(kt)
                                nc.scalar.activation(
                                    out=e_sb[:, lo : (j + 1) * QCH],
                                    in_=s_ps[pos][:, lo : (j + 1) * QCH],
                                    func=AF.Exp,
                                    scale=scale,
                                )
                            for j in range(KG):
                                kt = kg * KG + j
                                jd = kt - ch * KPC
                                if 0 <= jd < KPC:
                                    tri = slice(j * QCH + jd * P, j * QCH + (jd + 1) * P)
                                    nc.vector.tensor_mul(e_sb[:, tri], e_sb[:, tri], mask_sb)
                            for j in range(KG):
                                kt = kg * KG + j
                                lh = 2 * pr + pos
                                nc.tensor.matmul(
                                    att_ps[pos][0 : HD + 1, vis(kt) : QCH],
                                    lhsT=V4[kt][:, lh, :],
                                    rhs=e_sb[:, j * QCH + vis(kt) : (j + 1) * QCH],
                                    start=(kt == 0),
                                    stop=(kt == n_kt - 1),
                                )
                    # normalize: att / rowsum via outer-product broadcast
                    at = attp.tile([P, QCH], BF16, tag="attT", name="attT")
                    bc_ps = psX.tile([P, QCH], FP32, tag="x", name="bc")
                    for pos in range(2):
                        rsum = rsp.tile([1, QCH], BF16, tag="r", name="rsum")
                        nc.scalar.copy(rsum, att_ps[pos][HD : HD + 1, :])
                        nc.tensor.matmul(
                            bc_ps[64 * pos : 64 * pos + 64, :],
                            lhsT=ones_t,
                            rhs=rsum,
                            start=True,
                            stop=True,
                            tile_position=(0, 64 * pos),
                        )
                    rec_bc = recp.tile([P, QCH], FP32, tag="c", name="recbc")
                    nc.vector.reciprocal_approx_fast(out=rec_bc, in_=bc_ps)
                    for pos in range(2):
                        rows = slice(64 * pos, 64 * pos + 64)
                        nc.vector.tensor_mul(
                            at[rows, :], att_ps[pos][0:64, :], rec_bc[rows, :]
                        )
                    attT.append(at)

                # ===== proj partial (+bp/GC) -> rs_in, then ReduceScatter =====
                slot = 0 if ch < 2 else ch - 1  # chunks 0+1 share rs slot 0
                row0 = (ch * QCH) if ch < 2 else 0
                for tt in range(KPC):
                    for oc in range(2):
                        pj = psMain.tile([P, 512], FP32, tag="m", name="pj")
                        for pr in range(NPAIR):
                            nc.tensor.matmul(
                                pj,
                                lhsT=attT[pr][:, tt * P : (tt + 1) * P],
                                rhs=wp_sb[:, pr, oc * 512 : (oc + 1) * 512],
                                start=(pr == 0),
                                stop=(pr == NPAIR - 1),
                            )
                        pj_sb = pjp.tile([P, 512], BF16, tag="pj", name="pjsb")
                        nc.vector.tensor_tensor(
                            out=pj_sb, in0=pj,
                            in1=bp4_bc[:, oc * 512 : (oc + 1) * 512], op=ALU.add,
                        )
                        nc.sync.dma_start(
                            rs_in_t[slot][
                                row0 + tt * P : row0 + (tt + 1) * P,
                                oc * 512 : (oc + 1) * 512,
                            ],
                            pj_sb,
                        )
                if ch != 0:
                    nc.gpsimd.collective_compute(
                        "ReduceScatter",
                        ALU.add,
                        replica_groups=groups,
                        ins=[rs_in_t[slot][:].opt()],
                        outs=[rs_out_t[slot][:].opt()],
                    )

        # ======== Phase E (residual + LN2) + FFN, PSUM re-planned ========
        x2pb = [None] * NST
        hidT = None

        with contextlib.ExitStack() as ps_est:
            psF = ps_est.enter_context(tc.tile_pool(name="psF", bufs=2, space="PSUM"))
            psW2 = ps_est.enter_context(tc.tile_pool(name="psW2", bufs=1, space="PSUM"))

            def phase_e(st, wait_ms=None):
                if wait_ms is not None:
                    # keep the RS-dependent residual add from being scheduled
                    # ahead of attention's vector ops (head-of-line blocking)
                    with tc.tile_wait_until(wait_ms):
                        phase_e(st)
                    return
                r_t = tok.tile([P, C], BF16, tag="tb", name=f"rt{st}")
                if st < 2:
                    nc.sync.dma_start(r_t, rs_out_t[0][st * P : (st + 1) * P, :])
                else:
                    nc.sync.dma_start(r_t, rs_out_t[st - 1][:])
                xs_t = tok.tile([P, C], FP32, tag="tb", name=f"xst{st}")
                nc.sync.dma_start(xs_t, x_shard[st * P : (st + 1) * P, :])
                x2_t = tok.tile([P, C], FP32, tag="tb", name=f"x2t{st}")
                nc.vector.tensor_add(x2_t, xs_t, r_t)
                mv = ln_stats(x2_t, tok, "ln2")
                h2_bf = tok.tile([P, C], BF16, tag="tb", name=f"h2bf{st}")
                nc.vector.tensor_scalar(
                    out=h2_bf,
                    in0=x2_t,
                    scalar1=mv[:, 0:1],
                    scalar2=mv[:, 1:2],
                    op0=ALU.subtract,
                    op1=ALU.mult,
                )
                xb = x2p.tile([P, C], BF16, tag="x2pb", name=f"x2pb{st}")
                nc.vector.tensor_add(xb, x2_t, b2_bc)
                x2pb[st] = xb
                trans = psF.tile([P, NCc * P], BF16, tag="h", name=f"tr2{st}")
                for ic in range(NCc):
                    nc.tensor.transpose(
                        trans[:, ic * P : (ic + 1) * P],
                        h2_bf[:, ic * P : (ic + 1) * P],
                        ident,
                    )
                nc.vector.tensor_copy(
                    h2T[:, st, :, :].rearrange("p a b -> p (a b)"), trans
                )

            phase_e(0, wait_ms=0.17)
            phase_e(1, wait_ms=0.175)
            phase_e(2, wait_ms=0.23)

            # ---- FFN prologue (fills the last RS window): for ht < GA run W1
            # over tokens 0..W1A AND the 6 main W2 accumulations (they only
            # read those tokens). The last-token-tile work stays in the main
            # loop behind RS3.
            hidT = big.tile([P, NHT, TSH], BF16, tag="big", name="hidT")
            psW2_tiles = [None] * 6
            for tt in range(NST - 1):
                for oc in range(2):
                    k = tt * 2 + oc
                    psW2_tiles[k] = psW2.tile([P, 512], FP32, tag=f"w2a{k}", name=f"w2a{k}")
                    nc.tensor.matmul(
                        psW2_tiles[k],
                        lhsT=ident,
                        rhs=x2pb[tt][:, oc * 512 : (oc + 1) * 512],
                        start=True,
                        stop=False,
                    )
            w1t_tiles = [None] * GA
            for ht in range(GA):
                w1t = w1p.tile([P, NCc, P], BF16, tag="w1t", name=f"w1t{ht}")
                nc.sync.dma_start(
                    w1t,
                    w1_d[:, ht * P : (ht + 1) * P].rearrange("(n p) m -> p n m", p=P),
                )
                w1t_tiles[ht] = w1t
                hid_ps = psF.tile([P, W1A], FP32, tag="h", name=f"ha{ht}")
                for ic in range(NCc):
                    nc.tensor.matmul(
                        hid_ps,
                        lhsT=w1t[:, ic, :],
                        rhs=h2T[:, 0 : NST - 1, ic, :],
                        start=(ic == 0),
                        stop=(ic == NCc - 1),
                    )
                nc.scalar.activation(
                    out=hidT[:, ht, 0:W1A],
                    in_=hid_ps,
                    func=AF.Relu,
                    bias=b1_sb[:, ht : ht + 1],
                    scale=1.0,
                )
                w2t = w2p.tile([P, C], BF16, tag="w2t", name=f"w2p{ht}")
                nc.sync.dma_start(w2t, w2_d[ht * P : (ht + 1) * P, :])
                for tt in range(NST - 1):
                    for oc in range(2):
                        nc.tensor.matmul(
                            psW2_tiles[tt * 2 + oc],
                            lhsT=hidT[:, ht, tt * P : (tt + 1) * P],
                            rhs=w2t[:, oc * 512 : (oc + 1) * 512],
                            start=False,
                            stop=False,
                        )

            phase_e(NST - 1)

            # ---- FFN main loop ----
            W = 4  # spill window (hidden tiles per burst for the tt3 outputs)
            acc3 = [
                accp.tile([P, 512], FP32, tag=f"acc{oc}", name=f"acc{oc}")
                for oc in range(2)
            ]
            w2t_tiles = {}
            for ht in range(NHT):
                if ht % W == 0:
                    # stage this burst window's re-fetched w2 rows (prologue
                    # hidden tiles no longer have their w2t resident)
                    for hh in range(ht, min(ht + W, NHT)):
                        if hh < GA:
                            w2b = w2bp.tile([P, C], BF16, tag="w2b", name=f"w2b{hh}")
                            nc.sync.dma_start(w2b, w2_d[hh * P : (hh + 1) * P, :])
                            w2t_tiles[hh] = w2b
                if ht >= GA:
                    w2t = w2p.tile([P, C], BF16, tag="w2t", name=f"w2t{ht}")
                    nc.sync.dma_start(w2t, w2_d[ht * P : (ht + 1) * P, :])
                    w2t_tiles[ht] = w2t
                if ht < GA:
                    hid_ps = psF.tile([P, P], FP32, tag="h", name=f"hb{ht}")
                    for ic in range(NCc):
                        nc.tensor.matmul(
                            hid_ps,
                            lhsT=w1t_tiles[ht][:, ic, :],
                            rhs=h2T[:, NST - 1, ic, :],
                            start=(ic == 0),
                            stop=(ic == NCc - 1),
                        )
                    nc.scalar.activation(
                        out=hidT[:, ht, W1A:TSH],
                        in_=hid_ps,
                        func=AF.Relu,
                        bias=b1_sb[:, ht : ht + 1],
                        scale=1.0,
                    )
                else:
                    w1t = w1p.tile([P, NCc, P], BF16, tag="w1t", name=f"w1t{ht}")
                    nc.sync.dma_start(
                        w1t,
                        w1_d[:, ht * P : (ht + 1) * P].rearrange(
                            "(n p) m -> p n m", p=P
                        ),
                    )
                    hid_ps = psF.tile([P, TSH], FP32, tag="h", name=f"hf{ht}")
                    for ic in range(NCc):
                        nc.tensor.matmul(
                            hid_ps,
                            lhsT=w1t[:, ic, :],
                            rhs=h2T[:, :, ic, :],
                            start=(ic == 0),
                            stop=(ic == NCc - 1),
                        )
                    nc.scalar.activation(
                        out=hidT[:, ht, :],
                        in_=hid_ps,
                        func=AF.Relu,
                        bias=b1_sb[:, ht : ht + 1],
                        scale=1.0,
                    )
                # main W2 accumulation: token tiles 0..2 x both column halves
                # (prologue already did these for ht < GA)
                if ht >= GA:
                    for tt in range(NST - 1):
                        for oc in range(2):
                            k = tt * 2 + oc
                            nc.tensor.matmul(
                                psW2_tiles[k],
                                lhsT=hidT[:, ht, tt * P : (tt + 1) * P],
                                rhs=w2t[:, oc * 512 : (oc + 1) * 512],
                                start=False,
                                stop=(ht == NHT - 1),
                            )
                # windowed burst for the last token tile (both column halves)
                if (ht + 1) % W == 0:
                    h0 = ht + 1 - W
                    for oc in range(2):
                        bp_ps = psF.tile([P, 512], FP32, tag="h", name=f"b3{ht}{oc}")
                        if h0 == 0:
                            nc.tensor.matmul(
                                bp_ps,
                                lhsT=ident,
                                rhs=x2pb[NST - 1][:, oc * 512 : (oc + 1) * 512],
                                start=True,
                                stop=False,
                            )
                        for hh in range(h0, ht + 1):
                            nc.tensor.matmul(
                                bp_ps,
                                lhsT=hidT[:, hh, (NST - 1) * P : TSH],
                                rhs=w2t_tiles[hh][:, oc * 512 : (oc + 1) * 512],
                                start=False if h0 == 0 else (hh == h0),
                                stop=(hh == ht),
                            )
                        if h0 == 0:
                            nc.vector.tensor_copy(acc3[oc], bp_ps)
                        else:
                            nc.vector.tensor_add(acc3[oc], acc3[oc], bp_ps)
                    for hh in range(h0, ht + 1):
                        w2t_tiles.pop(hh)

            # ---- outputs: convert + store (residual already injected) ----
            for tt in range(NST - 1):
                for oc in range(2):
                    ob = obp.tile([P, 512], BF16, tag="ob", name="ob")
                    nc.scalar.copy(ob, psW2_tiles[tt * 2 + oc])
                    nc.sync.dma_start(
                        out_d[tt * P : (tt + 1) * P, oc * 512 : (oc + 1) * 512], ob
                    )
            for oc in range(2):
                ob = obp.tile([P, 512], BF16, tag="ob", name="ob")
                nc.scalar.copy(ob, acc3[oc])
                nc.sync.dma_start(
                    out_d[(NST - 1) * P : TSH, oc * 512 : (oc + 1) * 512], ob
                )

    nc.finalize()
    return nc


# ------------------------- host side -------------------------

_CACHE = {}
LAST_RESULTS = None


def make_in_maps(inputs, T=2048, C=1024, H=16, F=4096, GC=4, n_cores=8):
    HD = 64
    NHL = H // GC
    DL = NHL * HD
    NHT = F // P
    NPAIR = NHL // 2
    bf = ml_dtypes.bfloat16

    x = np.asarray(inputs["x"], np.float32)
    Wq = np.asarray(inputs["Wq"], np.float32)
    Wk = np.asarray(inputs["Wk"], np.float32)
    Wv = np.asarray(inputs["Wv"], np.float32)
    Wp = np.asarray(inputs["Wp"], np.float32)
    bp = np.asarray(inputs["bp"], np.float32)
    W1 = np.asarray(inputs["W1"], np.float32)
    b1 = np.asarray(inputs["b1"], np.float32)
    W2 = np.asarray(inputs["W2"], np.float32)
    b2 = np.asarray(inputs["b2"], np.float32)
    g1 = np.asarray(inputs["g1"], np.float32)
    be1 = np.asarray(inputs["beta1"], np.float32)
    g2 = np.asarray(inputs["g2"], np.float32)
    be2 = np.asarray(inputs["beta2"], np.float32)

    maskr = np.triu(np.ones((P, P), np.float32)).astype(bf)  # m[kr,qc]=kr<=qc
    b1_eff = b1 + be2 @ W1
    b1r = np.ascontiguousarray(b1_eff.reshape(NHT, P).T)
    w1b = (g2[:, None] * W1).astype(bf)
    w2b = W2.astype(bf)

    RPC = QCH // GC  # 128

    def shard_rows(g):
        # chunks 0+1 are ReduceScatter'd together (core g owns 256 contiguous
        # rows of q[0:1024]); chunks 2 and 3 are scattered separately.
        return np.concatenate(
            [
                np.arange(g * 2 * RPC, (g + 1) * 2 * RPC),
                np.arange(2 * QCH + g * RPC, 2 * QCH + (g + 1) * RPC),
                np.arange(3 * QCH + g * RPC, 3 * QCH + (g + 1) * RPC),
            ]
        )

    in_maps = []
    for c in range(n_cores):
        b, g = c // GC, c % GC
        hsl = slice(g * NHL, (g + 1) * NHL)
        Wq_g, Wk_g, Wv_g = Wq[hsl], Wk[hsl], Wv[hsl]  # [NHL, C, HD]
        qbias = np.einsum("c,hcd->hd", be1, Wq_g)
        kbias = np.einsum("c,hcd->hd", be1, Wk_g)
        vbias = np.einsum("c,hcd->hd", be1, Wv_g)
        qkb = np.zeros((P, 2 * NPAIR * 2), np.float32)
        for pr in range(NPAIR):
            qkb[:, pr] = np.concatenate([qbias[2 * pr], qbias[2 * pr + 1]])
            qkb[:, NPAIR + pr] = np.concatenate([kbias[2 * pr], kbias[2 * pr + 1]])
        vb = np.zeros((NHL * (HD + 1),), np.float32)
        for h in range(NHL):
            vb[h * (HD + 1) : h * (HD + 1) + HD] = vbias[h]
            vb[h * (HD + 1) + HD] = 1.0
        in_maps.append(
            {
                "x_full": np.ascontiguousarray(x[b]).astype(bf),
                "x_shard": np.ascontiguousarray(x[b][shard_rows(g)]),
                "wq": np.ascontiguousarray(
                    (g1[:, None, None] * Wq_g.transpose(1, 0, 2)).reshape(C, DL)
                ).astype(bf),
                "wk": np.ascontiguousarray(
                    (g1[:, None, None] * Wk_g.transpose(1, 0, 2)).reshape(C, DL)
                ).astype(bf),
                "wv": np.ascontiguousarray(
                    (g1[:, None, None] * Wv_g.transpose(1, 0, 2)).reshape(C, DL)
                ).astype(bf),
                "wp": np.ascontiguousarray(Wp[g * DL : (g + 1) * DL]).astype(bf),
                "w1": w1b,
                "w2": w2b,
                "qkb": qkb,
                "vb": vb.astype(bf),
                "b1r": b1r,
                "bp4": (bp / GC).astype(bf),
                "b2": b2.astype(bf),
                "maskr": maskr,
            }
        )
    return in_maps


def kernel(**inputs) -> np.ndarray:
    global LAST_RESULTS
    B, T, C = inputs["x"].shape
    H = inputs["Wq"].shape[0]
    F = inputs["W1"].shape[1]
    GC = 4
    n_cores = 8
    key = (T, C, H, F)
    if key not in _CACHE:
        _CACHE[key] = build_block(T=T, C=C, NHL=H // GC, F=F, GC=GC, n_cores=n_cores)
    nc = _CACHE[key]
    in_maps = make_in_maps(inputs, T=T, C=C, H=H, F=F, GC=GC, n_cores=n_cores)
    res = run_bass_kernel_spmd(nc, in_maps, core_ids=list(range(n_cores)))
    LAST_RESULTS = res
    out = np.empty((B, T, C), np.float32)
    RPC = QCH // GC
    for c in range(n_cores):
        b, g = c // GC, c % GC
        sh = np.asarray(res.results[c]["out"], dtype=np.float32)
        out[b, g * 2 * RPC : (g + 1) * 2 * RPC] = sh[0 : 2 * RPC]
        out[b, 2 * QCH + g * RPC : 2 * QCH + (g + 1) * RPC] = sh[2 * RPC : 3 * RPC]
        out[b, 3 * QCH + g * RPC : 3 * QCH + (g + 1) * RPC] = sh[3 * RPC : 4 * RPC]
    return out


# revision 42
# speedup vs baseline: 1.0903x; 1.0903x over previous
"""Trainium2 Bass kernel: dense transformer block (B=2, T=2048, C=1024, H=16, HD=64).

Sharding over 8 NeuronCores: 2 batch groups (data parallel over B) x 4-way
tensor parallel within each group. Per group of 4 cores:
  - attention: heads split 4 ways (4 heads/core); per-core proj partials are
    ReduceScatter'd (bf16) over the token axis.
  - FFN: token-split (512 tokens/core), full W1/W2 streamed per core in bf16.

v2: fused chunk pipeline (LN1 -> QKV -> attention -> proj -> RS per 512-token
q-chunk), LN gains/biases folded into weights on the host, bf16 PE transposes,
merged QKV head-pair matmuls, AV matmul with a fused ones-column rowsum,
FFN with split W1 (fills the last-RS window) and windowed PSUM spill so the
whole W2 accumulation fits 8 banks with no second pass.
"""

import os
import sys

if "/opt/trn_rl_repo" not in sys.path:
    sys.path.insert(0, "/opt/trn_rl_repo")

import contextlib
import math

import ml_dtypes
import numpy as np

import concourse.bass as bass
import concourse.mybir as mybir
import concourse.tile as tile
from concourse import bacc
from concourse.bass_utils import run_bass_kernel_spmd
from concourse.masks import make_identity

# bass_utils' trace path imports antenv.axon_hooks, absent in this container.
try:
    from antenv import axon_hooks as _ah  # noqa: F401
except ImportError:
    import types as _types

    _shim = _types.ModuleType("antenv.axon_hooks")
    _shim._hook = None
    _shim.set_axon_ntff_profile_hook = lambda h: setattr(_shim, "_hook", h)
    _shim.get_axon_ntff_profile_hook = lambda: _shim._hook
    sys.modules["antenv.axon_hooks"] = _shim
    try:
        if "/root/.axon_site" not in sys.path:
            sys.path.insert(0, "/root/.axon_site")
        from trn_agent_boot.trn_boot import _ntff_profile_via_ctypes

        _shim.set_axon_ntff_profile_hook(
            _ntff_profile_via_ctypes("/opt/axon/libaxon_pjrt.so")
        )
    except Exception:
        pass

AF = mybir.ActivationFunctionType
ALU = mybir.AluOpType
FP32 = mybir.dt.float32
BF16 = mybir.dt.bfloat16

P = 128
QCH = 512  # query chunk (free dim of S^T matmuls)
KG = 2  # k-tiles batched per exp() call
GA = 13  # hidden tiles whose W1 token-prefix runs during the last RS


def build_block(T=2048, C=1024, NHL=4, F=4096, GC=4, eps=1e-5, n_cores=8):
    """Emit the per-core SPMD program. NHL = local heads (64-dim each)."""
    HD = 64
    DL = NHL * HD  # 256
    NPAIR = NHL // 2
    NT = T // P  # token tiles (16)
    NCc = C // P  # channel tiles (8)
    NQC = T // QCH  # query chunks (4)
    KPC = QCH // P  # k-tiles per chunk (4)
    TPC = NT // NQC  # token tiles per chunk (4)
    TSH = T // GC  # token shard (512)
    NST = TSH // P  # shard token tiles (4)
    NHT = F // P  # FFN hidden tiles (32)
    W1A = (NST - 1) * P  # W1 token prefix computable before last RS (384)
    scale = 1.0 / math.sqrt(HD)

    groups = [list(range(g * GC, (g + 1) * GC)) for g in range(n_cores // GC)]

    nc = bacc.Bacc(
        "TRN2", target_bir_lowering=False, num_devices=n_cores, debug=False
    )

    # ---- I/O ----
    x_full = nc.dram_tensor("x_full", [T, C], BF16, kind="ExternalInput")
    x_shard = nc.dram_tensor("x_shard", [TSH, C], FP32, kind="ExternalInput")
    wq_d = nc.dram_tensor("wq", [C, DL], BF16, kind="ExternalInput")
    wk_d = nc.dram_tensor("wk", [C, DL], BF16, kind="ExternalInput")
    wv_d = nc.dram_tensor("wv", [C, DL], BF16, kind="ExternalInput")
    wp_d = nc.dram_tensor("wp", [DL, C], BF16, kind="ExternalInput")
    w1_d = nc.dram_tensor("w1", [C, F], BF16, kind="ExternalInput")
    w2_d = nc.dram_tensor("w2", [F, C], BF16, kind="ExternalInput")
    qkb_d = nc.dram_tensor("qkb", [P, 2 * NPAIR * 2], FP32, kind="ExternalInput")
    vb_d = nc.dram_tensor("vb", [NHL * (HD + 1)], BF16, kind="ExternalInput")
    b1_d = nc.dram_tensor("b1r", [P, NHT], FP32, kind="ExternalInput")
    bp4_d = nc.dram_tensor("bp4", [C], BF16, kind="ExternalInput")
    b2_d = nc.dram_tensor("b2", [C], BF16, kind="ExternalInput")
    mask_d = nc.dram_tensor("maskr", [P, P], BF16, kind="ExternalInput")
    out_d = nc.dram_tensor("out", [TSH, C], BF16, kind="ExternalOutput")

    def bcast(dst, src_ap):
        """DMA a [N]-shaped dram AP broadcast onto [P, N] sbuf tile."""
        nc.sync.dma_start(
            dst,
            bass.AP(
                tensor=src_ap.tensor,
                offset=src_ap.offset,
                ap=[[0, P]] + list(src_ap.ap),
            ),
        )

    with tile.TileContext(nc) as tc, contextlib.ExitStack() as est:
        sing = est.enter_context(tc.tile_pool(name="sing", bufs=1))
        big = est.enter_context(tc.tile_pool(name="big", bufs=1))
        tok = est.enter_context(tc.tile_pool(name="tok", bufs=6))
        xp = est.enter_context(tc.tile_pool(name="xp", bufs=8))
        vp = est.enter_context(tc.tile_pool(name="vp", bufs=NT))
        attp = est.enter_context(tc.tile_pool(name="attp", bufs=2 * NPAIR))
        exps = est.enter_context(tc.tile_pool(name="exps", bufs=3))
        rsp = est.enter_context(tc.tile_pool(name="rsp", bufs=4))
        recp = est.enter_context(tc.tile_pool(name="recp", bufs=2))
        pjp = est.enter_context(tc.tile_pool(name="pjp", bufs=4))
        x2p = est.enter_context(tc.tile_pool(name="x2p", bufs=NST))
        w1p = est.enter_context(tc.tile_pool(name="w1p", bufs=GA))
        w2p = est.enter_context(tc.tile_pool(name="w2p", bufs=6))
        accp = est.enter_context(tc.tile_pool(name="accp", bufs=1))
        obp = est.enter_context(tc.tile_pool(name="obp", bufs=3))
        dram = est.enter_context(tc.tile_pool(name="dram", bufs=1, space="DRAM"))

        # ---- x tile prefetch: issue DMAs well before the collectives start
        # hogging the fabric (a ReduceScatter starves ordinary DMA traffic).
        x_tiles = {}

        def fetch_x(it):
            if it in x_tiles:
                return
            t = xp.tile([P, C], BF16, tag="x", name=f"xt{it}")
            nc.sync.dma_start(t, x_full[it * P : (it + 1) * P, :])
            x_tiles[it] = t

        for it in range(2 * TPC):  # chunks 0+1 up front
            fetch_x(it)

        ident = sing.tile([P, P], BF16, tag="ident", name="ident")
        make_identity(nc, ident)
        eps_t = sing.tile([P, 1], FP32, tag="eps", name="eps")
        nc.vector.memset(eps_t, eps)
        ones_t = sing.tile([1, HD], BF16, tag="ones", name="ones")
        nc.vector.memset(ones_t, 1.0)

        wq_sb = sing.tile([P, NCc, DL], BF16, tag="wq", name="wq")
        nc.sync.dma_start(wq_sb, wq_d.rearrange("(n p) m -> p n m", p=P))
        wk_sb = sing.tile([P, NCc, DL], BF16, tag="wk", name="wk")
        nc.sync.dma_start(wk_sb, wk_d.rearrange("(n p) m -> p n m", p=P))
        wv_sb = sing.tile([P, NCc, DL], BF16, tag="wv", name="wv")
        nc.sync.dma_start(wv_sb, wv_d.rearrange("(n p) m -> p n m", p=P))
        wp_sb = sing.tile([P, DL // P, C], BF16, tag="wp", name="wp")
        nc.sync.dma_start(wp_sb, wp_d.rearrange("(n p) m -> p n m", p=P))
        mask_sb = sing.tile([P, P], BF16, tag="mask", name="mask")
        nc.sync.dma_start(mask_sb, mask_d[:])
        qkb_sb = sing.tile([P, 2 * NPAIR * 2], FP32, tag="qkb", name="qkb")
        nc.sync.dma_start(qkb_sb, qkb_d[:])
        vb_sb = sing.tile([P, NHL, HD + 1], BF16, tag="vb", name="vb")
        bcast(vb_sb.rearrange("p h d -> p (h d)"), vb_d[:])
        b1_sb = sing.tile([P, NHT], FP32, tag="b1", name="b1")
        nc.sync.dma_start(b1_sb, b1_d[:])
        bp4_bc = sing.tile([P, C], BF16, tag="bp4", name="bp4")
        bcast(bp4_bc, bp4_d[:])
        b2_bc = sing.tile([P, C], BF16, tag="b2bc", name="b2bc")
        bcast(b2_bc, b2_d[:])

        # hT (attention input, feature-major, strided by token tile) shares its
        # 32KB/partition slot with hidT (FFN hidden) — disjoint lifetimes.
        hT = big.tile([P, NT, NCc, P], BF16, tag="big", name="hT")
        QT = [sing.tile([P, T], BF16, tag=f"qt{pr}", name=f"qt{pr}") for pr in range(NPAIR)]
        KT = [sing.tile([P, T], BF16, tag=f"kt{pr}", name=f"kt{pr}") for pr in range(NPAIR)]
        h2T = sing.tile([P, NST, NCc, P], BF16, tag="h2T", name="h2T")

        # chunks 0+1 share one ReduceScatter (fewer collective launches);
        # chunks 2 and 3 keep their own so the tail RS stays small.
        rs_in_t = [
            dram.tile([2 * QCH, C], BF16, tag="rsinA", name="rsinA"),
            dram.tile([QCH, C], BF16, tag="rsin2", name="rsin2"),
            dram.tile([QCH, C], BF16, tag="rsin3", name="rsin3"),
        ]
        rs_out_t = [
            dram.tile([2 * QCH // GC, C], BF16, tag="rsoutA", name="rsoutA"),
            dram.tile([QCH // GC, C], BF16, tag="rsout2", name="rsout2"),
            dram.tile([QCH // GC, C], BF16, tag="rsout3", name="rsout3"),
        ]

        # ---- LayerNorm stats helper (token-major [P, C] tile -> mv[P,2]) ----
        fmax = math.gcd(512, C)
        nsub = C // fmax

        def ln_stats(x_t, pool, tagp):
            stats = pool.tile([P, nsub, 6], FP32, tag=f"{tagp}_st", name=f"{tagp}_st")
            xr = x_t.rearrange("p (n f) -> p n f", n=nsub)
            for s in range(nsub):
                nc.vector.bn_stats(out=stats[:, s, :], in_=xr[:, s, :])
            mv = pool.tile([P, 2], FP32, tag=f"{tagp}_mv", name=f"{tagp}_mv")
            nc.vector.bn_aggr(out=mv, in_=stats)
            nc.scalar.activation(
                out=mv[:, 1:2], in_=mv[:, 1:2], func=AF.Sqrt, bias=eps_t, scale=1.0
            )
            nc.vector.reciprocal(out=mv[:, 1:2], in_=mv[:, 1:2])
            return mv

        V4 = [None] * NT

        with contextlib.ExitStack() as ps_est:
            psMain = ps_est.enter_context(tc.tile_pool(name="psMain", bufs=2, space="PSUM"))
            psS = ps_est.enter_context(tc.tile_pool(name="psS", bufs=2, space="PSUM"))
            psX = ps_est.enter_context(tc.tile_pool(name="psX", bufs=2, space="PSUM"))

            def phase_avqkv(ch):
                """LN1 + transpose + V + QKV for one 512-token chunk."""
                if ch + 2 < NQC:  # stage next-next chunk's inputs now
                    for tt in range(TPC):
                        fetch_x((ch + 2) * TPC + tt)
                for tt in range(TPC):
                    it = ch * TPC + tt
                    x_t = x_tiles.pop(it)
                    mv = ln_stats(x_t, tok, "ln1")
                    h_bf = tok.tile([P, C], BF16, tag="tb", name=f"hbf{it}")
                    nc.vector.tensor_scalar(
                        out=h_bf,
                        in0=x_t,
                        scalar1=mv[:, 0:1],
                        scalar2=mv[:, 1:2],
                        op0=ALU.subtract,
                        op1=ALU.mult,
                    )
                    trans = psX.tile([P, NCc * P], BF16, tag="x", name=f"tr{it}")
                    for ic in range(NCc):
                        nc.tensor.transpose(
                            trans[:, ic * P : (ic + 1) * P],
                            h_bf[:, ic * P : (ic + 1) * P],
                            ident,
                        )
                    nc.vector.tensor_copy(
                        hT[:, it, :, :].rearrange("p a b -> p (a b)"), trans
                    )

                for tt in range(TPC):
                    it = ch * TPC + tt
                    ps = psMain.tile([P, DL], FP32, tag="m", name=f"vps{it}")
                    for ic in range(NCc):
                        nc.tensor.matmul(
                            ps,
                            lhsT=hT[:, it, ic, :],
                            rhs=wv_sb[:, ic, :],
                            start=(ic == 0),
                            stop=(ic == NCc - 1),
                        )
                    v_t = vp.tile([P, NHL, HD + 1], BF16, tag="v", name=f"v{it}")
                    nc.vector.tensor_tensor(
                        out=v_t[:, :, 0:HD],
                        in0=ps.rearrange("p (h d) -> p h d", h=NHL),
                        in1=vb_sb[:, :, 0:HD],
                        op=ALU.add,
                    )
                    nc.vector.tensor_copy(v_t[:, :, HD : HD + 1], vb_sb[:, :, HD : HD + 1])
                    V4[it] = v_t

                for pr in range(NPAIR):
                    for di, (dst, w_sb) in enumerate(((QT, wq_sb), (KT, wk_sb))):
                        ps = psMain.tile([P, QCH], FP32, tag="m", name=f"qk{pr}{di}")
                        for ic in range(NCc):
                            nc.tensor.matmul(
                                ps,
                                lhsT=w_sb[:, ic, pr * P : (pr + 1) * P],
                                rhs=hT[:, ch * TPC : (ch + 1) * TPC, ic, :],
                                start=(ic == 0),
                                stop=(ic == NCc - 1),
                            )
                        col = di * NPAIR + pr
                        dseg = dst[pr][:, ch * QCH : (ch + 1) * QCH]
                        nc.scalar.activation(
                            out=dseg, in_=ps, func=AF.Identity,
                            bias=qkb_sb[:, col : col + 1], scale=1.0,
                        )

            # software pipeline: LN/QKV of chunk ch+1 runs during attention(ch)
            phase_avqkv(0)
            for ch in range(NQC):
                if ch + 1 < NQC:
                    phase_avqkv(ch + 1)

                # ===== attention for this chunk =====
                n_kt = (ch + 1) * KPC
                attT = []
                for pr in range(NPAIR):
                    att_ps = [
                        psMain.tile([P, QCH], FP32, tag="m", name=f"av{ch}{pr}{i}")
                        for i in range(2)
                    ]
                    for kg in range(n_kt // KG):
                        s_ps = [
                            psS.tile([P, KG * QCH], FP32, tag="s", name=f"s{pr}{kg}{i}")
                            for i in range(2)
                        ]

                        def vis(kt):
                            """first causally-visible q column for this k-tile"""
                            jd = kt - ch * KPC
                            return jd * P if 0 < jd < KPC else 0

                        for j in range(KG):
                            kt = kg * KG + j
                            for pos in range(2):
                                nc.tensor.matmul(
                                    s_ps[pos][:, j * QCH + vis(kt) : (j + 1) * QCH],
                                    lhsT=KT[pr][
                                        64 * pos : 64 * pos + 64, kt * P : (kt + 1) * P
                                    ],
                                    rhs=QT[pr][
                                        64 * pos : 64 * pos + 64,
                                        ch * QCH + vis(kt) : (ch + 1) * QCH,
                                    ],
                                    start=True,
                                    stop=True,
                                    tile_position=(64 * pos, 0),
                                )
                        for pos in range(2):
                            e_sb = exps.tile([P, KG * QCH], BF16, tag="e", name="e")
                            for j in range(KG):
                                kt = kg * KG + j
                                lo = j * QCH + vis(kt)
                                nc.scalar.activation(
                                    out=e_sb[:, lo : (j + 1) * QCH],
                                    in_=s_ps[pos][:, lo : (j + 1) * QCH],
                                    func=AF.Exp,
                                    scale=scale,
                                )
                            for j in range(KG):
                                kt = kg * KG + j
                                jd = kt - ch * KPC
                                if 0 <= jd < KPC:
                                    tri = slice(j * QCH + jd * P, j * QCH + (jd + 1) * P)
                                    nc.vector.tensor_mul(e_sb[:, tri], e_sb[:, tri], mask_sb)
                            for j in range(KG):
                                kt = kg * KG + j
                                lh = 2 * pr + pos
                                nc.tensor.matmul(
                                    att_ps[pos][0 : HD + 1, vis(kt) : QCH],
                                    lhsT=V4[kt][:, lh, :],
                                    rhs=e_sb[:, j * QCH + vis(kt) : (j + 1) * QCH],
                                    start=(kt == 0),
                                    stop=(kt == n_kt - 1),
                                )
                    # normalize: att / rowsum via outer-product broadcast
                    at = attp.tile([P, QCH], BF16, tag="attT", name="attT")
                    bc_ps = psX.tile([P, QCH], FP32, tag="x", name="bc")
                    for pos in range(2):
                        rsum = rsp.tile([1, QCH], BF16, tag="r", name="rsum")
                        nc.scalar.copy(rsum, att_ps[pos][HD : HD + 1, :])
                        nc.tensor.matmul(
                            bc_ps[64 * pos : 64 * pos + 64, :],
                            lhsT=ones_t,
                            rhs=rsum,
                            start=True,
                            stop=True,
                            tile_position=(0, 64 * pos),
                        )
                    rec_bc = recp.tile([P, QCH], FP32, tag="c", name="recbc")
                    nc.vector.reciprocal_approx_fast(out=rec_bc, in_=bc_ps)
                    for pos in range(2):
                        rows = slice(64 * pos, 64 * pos + 64)
                        nc.vector.tensor_mul(
                            at[rows, :], att_ps[pos][0:64, :], rec_bc[rows, :]
                        )
                    attT.append(at)

                # ===== proj partial (+bp/GC) -> rs_in, then ReduceScatter =====
                slot = 0 if ch < 2 else ch - 1  # chunks 0+1 share rs slot 0
                row0 = (ch * QCH) if ch < 2 else 0
                for tt in range(KPC):
                    for oc in range(2):
                        pj = psMain.tile([P, 512], FP32, tag="m", name="pj")
                        for pr in range(NPAIR):
                            nc.tensor.matmul(
                                pj,
                                lhsT=attT[pr][:, tt * P : (tt + 1) * P],
                                rhs=wp_sb[:, pr, oc * 512 : (oc + 1) * 512],
                                start=(pr == 0),
                                stop=(pr == NPAIR - 1),
                            )
                        pj_sb = pjp.tile([P, 512], BF16, tag="pj", name="pjsb")
                        nc.vector.tensor_tensor(
                            out=pj_sb, in0=pj,
                            in1=bp4_bc[:, oc * 512 : (oc + 1) * 512], op=ALU.add,
                        )
                        nc.sync.dma_start(
                            rs_in_t[slot][
                                row0 + tt * P : row0 + (tt + 1) * P,
                                oc * 512 : (oc + 1) * 512,
                            ],
                            pj_sb,
                        )
                if ch != 0:
                    nc.gpsimd.collective_compute(
                        "ReduceScatter",
                        ALU.add,
                        replica_groups=groups,
                        ins=[rs_in_t[slot][:].opt()],
                        outs=[rs_out_t[slot][:].opt()],
                    )

        # ======== Phase E (residual + LN2) + FFN, PSUM re-planned ========
        x2pb = [None] * NST
        hidT = None

        with contextlib.ExitStack() as ps_est:
            psF = ps_est.enter_context(tc.tile_pool(name="psF", bufs=2, space="PSUM"))
            psW2 = ps_est.enter_context(tc.tile_pool(name="psW2", bufs=1, space="PSUM"))

            def phase_e(st, wait_ms=None):
                if wait_ms is not None:
                    # keep the RS-dependent residual add from being scheduled
                    # ahead of attention's vector ops (head-of-line blocking)
                    with tc.tile_wait_until(wait_ms):
                        phase_e(st)
                    return
                r_t = tok.tile([P, C], BF16, tag="tb", name=f"rt{st}")
                if st < 2:
                    nc.sync.dma_start(r_t, rs_out_t[0][st * P : (st + 1) * P, :])
                else:
                    nc.sync.dma_start(r_t, rs_out_t[st - 1][:])
                xs_t = tok.tile([P, C], FP32, tag="tb", name=f"xst{st}")
                nc.sync.dma_start(xs_t, x_shard[st * P : (st + 1) * P, :])
                x2_t = tok.tile([P, C], FP32, tag="tb", name=f"x2t{st}")
                nc.vector.tensor_add(x2_t, xs_t, r_t)
                mv = ln_stats(x2_t, tok, "ln2")
                h2_bf = tok.tile([P, C], BF16, tag="tb", name=f"h2bf{st}")
                nc.vector.tensor_scalar(
                    out=h2_bf,
                    in0=x2_t,
                    scalar1=mv[:, 0:1],
                    scalar2=mv[:, 1:2],
                    op0=ALU.subtract,
                    op1=ALU.mult,
                )
                xb = x2p.tile([P, C], BF16, tag="x2pb", name=f"x2pb{st}")
                nc.vector.tensor_add(xb, x2_t, b2_bc)
                x2pb[st] = xb
                trans = psF.tile([P, NCc * P], BF16, tag="h", name=f"tr2{st}")
                for ic in range(NCc):
                    nc.tensor.transpose(
                        trans[:, ic * P : (ic + 1) * P],
                        h2_bf[:, ic * P : (ic + 1) * P],
                        ident,
                    )
                nc.vector.tensor_copy(
                    h2T[:, st, :, :].rearrange("p a b -> p (a b)"), trans
                )

            phase_e(0, wait_ms=0.17)
            phase_e(1, wait_ms=0.175)
            phase_e(2, wait_ms=0.23)

            # ---- FFN prologue: W1 over tokens 0..W1A for ht < GA (fills RS3) ----
            hidT = big.tile([P, NHT, TSH], BF16, tag="big", name="hidT")
            w1t_tiles = [None] * GA
            for ht in range(GA):
                w1t = w1p.tile([P, NCc, P], BF16, tag="w1t", name=f"w1t{ht}")
                nc.sync.dma_start(
                    w1t,
                    w1_d[:, ht * P : (ht + 1) * P].rearrange("(n p) m -> p n m", p=P),
                )
                w1t_tiles[ht] = w1t
                hid_ps = psF.tile([P, W1A], FP32, tag="h", name=f"ha{ht}")
                for ic in range(NCc):
                    nc.tensor.matmul(
                        hid_ps,
                        lhsT=w1t[:, ic, :],
                        rhs=h2T[:, 0 : NST - 1, ic, :],
                        start=(ic == 0),
                        stop=(ic == NCc - 1),
                    )
                nc.scalar.activation(
                    out=hidT[:, ht, 0:W1A],
                    in_=hid_ps,
                    func=AF.Relu,
                    bias=b1_sb[:, ht : ht + 1],
                    scale=1.0,
                )

            phase_e(NST - 1)

            # ---- FFN main loop ----
            W = 4  # spill window (hidden tiles per burst for the tt3 outputs)
            acc3 = [
                accp.tile([P, 512], FP32, tag=f"acc{oc}", name=f"acc{oc}")
                for oc in range(2)
            ]
            # seed each W2 accumulator with the residual (x2 + b2) via an
            # identity matmul so no epilogue add is needed.
            psW2_tiles = [None] * 6
            for tt in range(NST - 1):
                for oc in range(2):
                    k = tt * 2 + oc
                    psW2_tiles[k] = psW2.tile([P, 512], FP32, tag=f"w2a{k}", name=f"w2a{k}")
                    nc.tensor.matmul(
                        psW2_tiles[k],
                        lhsT=ident,
                        rhs=x2pb[tt][:, oc * 512 : (oc + 1) * 512],
                        start=True,
                        stop=False,
                    )
            w2t_tiles = {}
            for ht in range(NHT):
                w2t = w2p.tile([P, C], BF16, tag="w2t", name=f"w2t{ht}")
                nc.sync.dma_start(w2t, w2_d[ht * P : (ht + 1) * P, :])
                w2t_tiles[ht] = w2t
                if ht < GA:
                    hid_ps = psF.tile([P, P], FP32, tag="h", name=f"hb{ht}")
                    for ic in range(NCc):
                        nc.tensor.matmul(
                            hid_ps,
                            lhsT=w1t_tiles[ht][:, ic, :],
                            rhs=h2T[:, NST - 1, ic, :],
                            start=(ic == 0),
                            stop=(ic == NCc - 1),
                        )
                    nc.scalar.activation(
                        out=hidT[:, ht, W1A:TSH],
                        in_=hid_ps,
                        func=AF.Relu,
                        bias=b1_sb[:, ht : ht + 1],
                        scale=1.0,
                    )
                else:
                    w1t = w1p.tile([P, NCc, P], BF16, tag="w1t", name=f"w1t{ht}")
                    nc.sync.dma_start(
                        w1t,
                        w1_d[:, ht * P : (ht + 1) * P].rearrange(
                            "(n p) m -> p n m", p=P
                        ),
                    )
                    hid_ps = psF.tile([P, TSH], FP32, tag="h", name=f"hf{ht}")
                    for ic in range(NCc):
                        nc.tensor.matmul(
                            hid_ps,
                            lhsT=w1t[:, ic, :],
                            rhs=h2T[:, :, ic, :],
                            start=(ic == 0),
                            stop=(ic == NCc - 1),
                        )
                    nc.scalar.activation(
                        out=hidT[:, ht, :],
                        in_=hid_ps,
                        func=AF.Relu,
                        bias=b1_sb[:, ht : ht + 1],
                        scale=1.0,
                    )
                # main W2 accumulation: token tiles 0..2 x both column halves
                for tt in range(NST - 1):
                    for oc in range(2):
                        k = tt * 2 + oc
                        nc.tensor.matmul(
                            psW2_tiles[k],
                            lhsT=hidT[:, ht, tt * P : (tt + 1) * P],
                            rhs=w2t[:, oc * 512 : (oc + 1) * 512],
                            start=False,
                            stop=(ht == NHT - 1),
                        )
                # windowed burst for the last token tile (both column halves)
                if (ht + 1) % W == 0:
                    h0 = ht + 1 - W
                    for oc in range(2):
                        bp_ps = psF.tile([P, 512], FP32, tag="h", name=f"b3{ht}{oc}")
                        if h0 == 0:
                            nc.tensor.matmul(
                                bp_ps,
                                lhsT=ident,
                                rhs=x2pb[NST - 1][:, oc * 512 : (oc + 1) * 512],
                                start=True,
                                stop=False,
                            )
                        for hh in range(h0, ht + 1):
                            nc.tensor.matmul(
                                bp_ps,
                                lhsT=hidT[:, hh, (NST - 1) * P : TSH],
                                rhs=w2t_tiles[hh][:, oc * 512 : (oc + 1) * 512],
                                start=False if h0 == 0 else (hh == h0),
                                stop=(hh == ht),
                            )
                        if h0 == 0:
                            nc.vector.tensor_copy(acc3[oc], bp_ps)
                        else:
                            nc.vector.tensor_add(acc3[oc], acc3[oc], bp_ps)
                    for hh in range(h0, ht + 1):
                        w2t_tiles.pop(hh)

            # ---- outputs: convert + store (residual already injected) ----
            for tt in range(NST - 1):
                for oc in range(2):
                    ob = obp.tile([P, 512], BF16, tag="ob", name="ob")
                    nc.scalar.copy(ob, psW2_tiles[tt * 2 + oc])
                    nc.sync.dma_start(
                        out_d[tt * P : (tt + 1) * P, oc * 512 : (oc + 1) * 512], ob
                    )
            for oc in range(2):
                ob = obp.tile([P, 512], BF16, tag="ob", name="ob")
                nc.scalar.copy(ob, acc3[oc])
                nc.sync.dma_start(
                    out_d[(NST - 1) * P : TSH, oc * 512 : (oc + 1) * 512], ob
                )

    nc.finalize()
    return nc


# ------------------------- host side -------------------------

_CACHE = {}
LAST_RESULTS = None


def make_in_maps(inputs, T=2048, C=1024, H=16, F=4096, GC=4, n_cores=8):
    HD = 64
    NHL = H // GC
    DL = NHL * HD
    NHT = F // P
    NPAIR = NHL // 2
    bf = ml_dtypes.bfloat16

    x = np.asarray(inputs["x"], np.float32)
    Wq = np.asarray(inputs["Wq"], np.float32)
    Wk = np.asarray(inputs["Wk"], np.float32)
    Wv = np.asarray(inputs["Wv"], np.float32)
    Wp = np.asarray(inputs["Wp"], np.float32)
    bp = np.asarray(inputs["bp"], np.float32)
    W1 = np.asarray(inputs["W1"], np.float32)
    b1 = np.asarray(inputs["b1"], np.float32)
    W2 = np.asarray(inputs["W2"], np.float32)
    b2 = np.asarray(inputs["b2"], np.float32)
    g1 = np.asarray(inputs["g1"], np.float32)
    be1 = np.asarray(inputs["beta1"], np.float32)
    g2 = np.asarray(inputs["g2"], np.float32)
    be2 = np.asarray(inputs["beta2"], np.float32)

    maskr = np.triu(np.ones((P, P), np.float32)).astype(bf)  # m[kr,qc]=kr<=qc
    b1_eff = b1 + be2 @ W1
    b1r = np.ascontiguousarray(b1_eff.reshape(NHT, P).T)
    w1b = (g2[:, None] * W1).astype(bf)
    w2b = W2.astype(bf)

    RPC = QCH // GC  # 128

    def shard_rows(g):
        # chunks 0+1 are ReduceScatter'd together (core g owns 256 contiguous
        # rows of q[0:1024]); chunks 2 and 3 are scattered separately.
        return np.concatenate(
            [
                np.arange(g * 2 * RPC, (g + 1) * 2 * RPC),
                np.arange(2 * QCH + g * RPC, 2 * QCH + (g + 1) * RPC),
                np.arange(3 * QCH + g * RPC, 3 * QCH + (g + 1) * RPC),
            ]
        )

    in_maps = []
    for c in range(n_cores):
        b, g = c // GC, c % GC
        hsl = slice(g * NHL, (g + 1) * NHL)
        Wq_g, Wk_g, Wv_g = Wq[hsl], Wk[hsl], Wv[hsl]  # [NHL, C, HD]
        qbias = np.einsum("c,hcd->hd", be1, Wq_g)
        kbias = np.einsum("c,hcd->hd", be1, Wk_g)
        vbias = np.einsum("c,hcd->hd", be1, Wv_g)
        qkb = np.zeros((P, 2 * NPAIR * 2), np.float32)
        for pr in range(NPAIR):
            qkb[:, pr] = np.concatenate([qbias[2 * pr], qbias[2 * pr + 1]])
            qkb[:, NPAIR + pr] = np.concatenate([kbias[2 * pr], kbias[2 * pr + 1]])
        vb = np.zeros((NHL * (HD + 1),), np.float32)
        for h in range(NHL):
            vb[h * (HD + 1) : h * (HD + 1) + HD] = vbias[h]
            vb[h * (HD + 1) + HD] = 1.0
        in_maps.append(
            {
                "x_full": np.ascontiguousarray(x[b]).astype(bf),
                "x_shard": np.ascontiguousarray(x[b][shard_rows(g)]),
                "wq": np.ascontiguousarray(
                    (g1[:, None, None] * Wq_g.transpose(1, 0, 2)).reshape(C, DL)
                ).astype(bf),
                "wk": np.ascontiguousarray(
                    (g1[:, None, None] * Wk_g.transpose(1, 0, 2)).reshape(C, DL)
                ).astype(bf),
                "wv": np.ascontiguousarray(
                    (g1[:, None, None] * Wv_g.transpose(1, 0, 2)).reshape(C, DL)
                ).astype(bf),
                "wp": np.ascontiguousarray(Wp[g * DL : (g + 1) * DL]).astype(bf),
                "w1": w1b,
                "w2": w2b,
                "qkb": qkb,
                "vb": vb.astype(bf),
                "b1r": b1r,
                "bp4": (bp / GC).astype(bf),
                "b2": b2.astype(bf),
                "maskr": maskr,
            }
        )
    return in_maps


def kernel(**inputs) -> np.ndarray:
    global LAST_RESULTS
    B, T, C = inputs["x"].shape
    H = inputs["Wq"].shape[0]
    F = inputs["W1"].shape[1]
    GC = 4
    n_cores = 8
    key = (T, C, H, F)
    if key not in _CACHE:
        _CACHE[key] = build_block(T=T, C=C, NHL=H // GC, F=F, GC=GC, n_cores=n_cores)
    nc = _CACHE[key]
    in_maps = make_in_maps(inputs, T=T, C=C, H=H, F=F, GC=GC, n_cores=n_cores)
    res = run_bass_kernel_spmd(nc, in_maps, core_ids=list(range(n_cores)))
    LAST_RESULTS = res
    out = np.empty((B, T, C), np.float32)
    RPC = QCH // GC
    for c in range(n_cores):
        b, g = c // GC, c % GC
        sh = np.asarray(res.results[c]["out"], dtype=np.float32)
        out[b, g * 2 * RPC : (g + 1) * 2 * RPC] = sh[0 : 2 * RPC]
        out[b, 2 * QCH + g * RPC : 2 * QCH + (g + 1) * RPC] = sh[2 * RPC : 3 * RPC]
        out[b, 3 * QCH + g * RPC : 3 * QCH + (g + 1) * RPC] = sh[3 * RPC : 4 * RPC]
    return out
